# revision 17
# baseline (speedup 1.0000x reference)
"""Fused multi-head-size-1 attention kernel for Trainium2 (Bass/Tile).

Problem: out = softmax((x_q Wq^T + bq)(x_k Wk^T + bk)^T / sqrt(D)) (x_v Wv^T + bv)
Shapes: B=8, QL=KL=2048, D=1024, fp32 in/out.

Sharding: data-parallel over batch. Core i processes batch i end-to-end;
no collectives.

Algebraic restructure (v2): expand the score product
  S = (xq Wq^T + 1 bq^T)(Wk xk^T + bk 1^T)
    = xq (Wq^T Wk) xk^T  +  (xq Wq^T bk) 1^T  +  1 (bq^T Wk xk^T)  +  (bq.bk) 1 1^T
The 2nd and 4th terms are constant along the key axis, so they drop out of
the softmax. Host precomputes M = Wq^T Wk (one 1024^3 matmul shared by all
batches) and c = (xk (Wk^T bq)) / 32 (a per-key bias folded into the exp
activation's per-partition bias operand). This removes the Q and K
projections entirely: device work drops from 30.1 to 25.8 GFLOP/core and
2032 -> 1792 matmul instructions.

Layout (v2.1): all DRAM operands are pre-packed host-side so every SBUF
partition's data is one contiguous DRAM run (8-32KB DMA descriptors; the
HWDGE queues are descriptor-rate-bound, so 2KB descriptors move at only
~110GB/s/queue while 8KB+ descriptors approach wire speed). A short PE
warm-up on scratch data during the DMA preamble absorbs the tensor
engine's p-state ramp.

Per-core dataflow (everything resident in SBUF in bf16):
  phase 1: XM^T[d2,q] = sum_d1 M[d1,d2]^T xq^T[d1,q]  (256 mm)
           V[k',h]   = xv Wv^T + bv, ones col appended (256 mm)
           xk^T is DMA'd straight to SBUF (no compute).
  phase 2: per q-block (512): S^T[k',q] = sum_d xk-tile^T XM^T (128 mm),
           P^T = exp(S^T/32 + c) via ScalarE (bf16, bias=c per partition),
           O[q,h] (+l) = P^T.T V_aug (192 mm, PSUM fp32),
           O = O * (1/l), DMA out. bv flows through the normalization
           (sum_k p_k/l = 1) so no epilogue bias is needed.
"""

import numpy as np
import ml_dtypes

import concourse.bass as bass
import concourse.mybir as mybir
from concourse.bacc import Bacc
from concourse.tile import TileContext
from concourse.bass_utils import run_bass_kernel_spmd

B, QL, KL, D = 8, 2048, 2048, 1024
P = 128
NCORES = 8
DT = D // P          # 8 tiles along d
KT = KL // P         # 16 tiles along k'
XCH = 512            # x streaming chunk along s
QC = QL // XCH       # 4 chunks
QB = 512             # q block for the attention stage
F32 = mybir.dt.float32
BF16 = mybir.dt.bfloat16
SCALE = 1.0 / 32.0   # 1/sqrt(D)

# AV free-dim chunking over V's 1025 columns (1024 h + ones column for l).
# The l-carrying chunk goes first so the reciprocal overlaps the other
# chunks' matmuls.
AV_CHUNKS = [(684, 1025), (0, 342), (342, 684)]
AV_MAXW = 342


def build_bass() -> bass.Bass:
    # Bacc (not bare Bass): its finalize() runs the pass pipeline that splits
    # multi-semaphore waits into event semaphores (TRN2 allows 1 wait/inst).
    nc = Bacc()

    # All operands pre-packed [partition, ...contiguous per partition].
    xq4 = nc.declare_dram_parameter("xq4", [P, QC, DT, XCH], BF16, isOutput=False)
    xkp = nc.declare_dram_parameter("xkp", [P, DT, KL], BF16, isOutput=False)
    xv4 = nc.declare_dram_parameter("xv4", [P, QC, DT, XCH], BF16, isOutput=False)
    m = nc.declare_dram_parameter("m", [P, DT, D], BF16, isOutput=False)
    wvp = nc.declare_dram_parameter("wvp", [P, DT, D], BF16, isOutput=False)
    cb = nc.declare_dram_parameter("cb", [P, KT], F32, isOutput=False)
    bv = nc.declare_dram_parameter("bv", [D], F32, isOutput=False)
    out = nc.declare_dram_parameter("out", [QL, D], F32, isOutput=True)

    with TileContext(nc) as tc:
        with (
            tc.tile_pool(name="persist", bufs=1) as persist,
            tc.tile_pool(name="consts", bufs=1) as consts,
        ):
            xk_sb = persist.tile([P, DT, KL], BF16, tag="xk")     # xk^T[d%128, dt, k]
            xmt_sb = persist.tile([P, DT, QL], BF16, tag="xmt")   # XM^T[d2%128, d2t, q]
            v_sb = persist.tile([P, KT, D + 1], BF16, tag="v")    # V[k'%128, kt, h|1]

            cb_sb = consts.tile([P, KT], F32, tag="cb")
            bv_sb = consts.tile([P, D], F32, tag="bv")
            warm = consts.tile([P, P], BF16, tag="warm")

            # ---------------- phase 1: XM + V projection ----------------
            with (
                tc.tile_pool(name="wpool", bufs=1) as wpool,
                tc.tile_pool(name="xpool", bufs=6) as xpool,
                tc.tile_pool(name="projp", bufs=8, space="PSUM") as projp,
            ):
                # PE warm-up on scratch zeros while the first DMAs land: the
                # tensor engine ramps 0.65 -> 1.2 -> 2.4 GHz over ~3us of
                # continuous work, so burn the ramp before real matmuls.
                nc.vector.memset(warm[:], 0.0)
                wps = projp.tile([P, XCH], F32, tag="proj")
                for _ in range(120):
                    nc.tensor.matmul(wps[:, 0:P], lhsT=warm[:], rhs=warm[:],
                                     start=True, stop=True)

                # Critical-path DMAs. Queue roles (measured rates differ a
                # lot, and descriptor size matters: <=4KB runs at half rate):
                # M split 5/8 on the fast gpsimd queue and 3/8 on scalar,
                # each as ONE big-descriptor DMA; xq0 in halves on SP's
                # queue. The slow scalar queue otherwise only carries data
                # needed late (Wv, cb). xpool bufs=6 so no x-chunk DMA
                # trigger ever blocks its engine waiting for a buffer.
                # M quartered across the two hardware-DGE queues (SP + ACT,
                # which spin up ~4us sooner than gpsimd's software DGE),
                # interleaved so each quarter lands before the d1t-outer
                # sweep consumes it.
                m_sb = wpool.tile([P, DT, D], BF16, tag="m")
                nc.sync.dma_start(out=m_sb[:, 0:2, :], in_=m[:, 0:2, :])
                nc.scalar.dma_start(out=m_sb[:, 2:4, :], in_=m[:, 2:4, :])
                nc.sync.dma_start(out=m_sb[:, 4:6, :], in_=m[:, 4:6, :])
                nc.scalar.dma_start(out=m_sb[:, 6:, :], in_=m[:, 6:, :])

                # XM^T: out[d2-tile, q-chunk] = sum_d1 M[d1,d2-tile]^T @ xq^T[d1,q-chunk]
                # d1t-outer with 8 concurrent PSUM groups: the PE consumes M
                # and xq slice-by-slice as they stream in. xq1 rides the same
                # queue as M's first half so a slow queue shifts both
                # together instead of opening a pipeline bubble.
                xcs = []
                for qc in range(QC):
                    xc = xpool.tile([P, DT, XCH], BF16, tag="x", name=f"xc{qc}")
                    xcs.append(xc)
                for dd in range(0, DT, 4):  # xq0 halves on gpsimd's SWDGE
                    nc.gpsimd.dma_start(out=xcs[0][:, dd:dd + 4, :],
                                        in_=xq4[:, 0, dd:dd + 4, :])
                nc.sync.dma_start(out=xcs[1][:], in_=xq4[:, 1, :, :])
                nc.sync.dma_start(out=xcs[2][:], in_=xq4[:, 2, :, :])
                nc.sync.dma_start(out=xcs[3][:], in_=xq4[:, 3, :, :])
                for qc in range(QC):
                    xc = xcs[qc]
                    pss = [projp.tile([P, XCH], F32, tag="proj", name=f"ps{qc}_{i}")
                           for i in range(DT)]
                    for d1t in range(DT):
                        for d2t in range(DT):
                            nc.tensor.matmul(
                                pss[d2t][:],
                                lhsT=m_sb[:, d1t, d2t * P:(d2t + 1) * P],
                                rhs=xc[:, d1t, :],
                                start=(d1t == 0),
                                stop=(d1t == DT - 1),
                            )
                    for d2t in range(DT):
                        nc.any.tensor_copy(
                            xmt_sb[:, d2t, qc * XCH:(qc + 1) * XCH], pss[d2t][:]
                        )

                # background DMAs for later stages, in need-order
                wv_sb = wpool.tile([P, DT, D], BF16, tag="wv")
                nc.scalar.dma_start(out=wv_sb[:], in_=wvp[:])
                nc.gpsimd.dma_start(out=xk_sb[:], in_=xkp[:])
                nc.scalar.dma_start(out=cb_sb[:], in_=cb[:])
                # broadcast bv across all partitions (stride-0 partition AP -> SWDGE)
                bv_bcast = bass.AP(tensor=bv[:].tensor, offset=0, ap=[[0, P], [1, D]])
                nc.gpsimd.dma_start(out=bv_sb[:], in_=bv_bcast)

                # V: out[s-tile, h-chunk] = sum_dt xv^T[d,s-tile]^T @ Wv^T[d,h-chunk]
                # + bv (broadcast over rows), fused into the PSUM->SBUF move.
                for cc in range(QC):
                    xc = xpool.tile([P, DT, XCH], BF16, tag="x")
                    nc.gpsimd.dma_start(out=xc[:], in_=xv4[:, cc, :, :])
                    for st4 in range(XCH // P):
                        st = cc * (XCH // P) + st4
                        for hc in range(D // 512):
                            ps = projp.tile([P, 512], F32, tag="proj")
                            for dt in range(DT):
                                nc.tensor.matmul(
                                    ps[:],
                                    lhsT=xc[:, dt, st4 * P:(st4 + 1) * P],
                                    rhs=wv_sb[:, dt, hc * 512:(hc + 1) * 512],
                                    start=(dt == 0),
                                    stop=(dt == DT - 1),
                                )
                            nc.any.tensor_add(
                                out=v_sb[:, st, hc * 512:(hc + 1) * 512],
                                in0=ps[:],
                                in1=bv_sb[:, hc * 512:(hc + 1) * 512],
                            )
                nc.vector.memset(v_sb[:, :, D], 1.0)  # ones column -> row sums

            # ---------------- phase 2: attention ----------------
            with (
                tc.tile_pool(name="ptpool", bufs=2) as ptpool,
                tc.tile_pool(name="opool", bufs=3) as opool,
                tc.tile_pool(name="small", bufs=4) as small,
                tc.tile_pool(name="scorep", bufs=3, space="PSUM") as scorep,
                tc.tile_pool(name="avp", bufs=4, space="PSUM") as avp,
            ):
                for qb in range(QL // QB):
                    q0 = qb * QB
                    ptb = ptpool.tile([P, KT, QB], BF16, tag="pt")
                    # scores S^T[k', q] = sum_dt xk-tile^T @ XM^T; exp folds
                    # in the per-key bias c (constant-per-query terms of the
                    # full score expansion drop out of softmax).
                    for kt in range(KT):
                        sp = scorep.tile([P, QB], F32, tag="score")
                        for dt in range(DT):
                            nc.tensor.matmul(
                                sp[:],
                                lhsT=xk_sb[:, dt, kt * P:(kt + 1) * P],
                                rhs=xmt_sb[:, dt, q0:q0 + QB],
                                start=(dt == 0),
                                stop=(dt == DT - 1),
                            )
                        nc.scalar.activation(
                            out=ptb[:, kt, :],
                            in_=sp[:],
                            func=mybir.ActivationFunctionType.Exp,
                            scale=SCALE,
                            bias=cb_sb[:, kt:kt + 1],
                        )
                    # AV + row sums + normalize, one q-tile (128 rows) at a time.
                    for qt4 in range(QB // P):
                        qrow = q0 + qt4 * P
                        rl = small.tile([P, 1], F32, tag="rl")
                        ob = opool.tile([P, D], F32, tag="o")
                        for ci, (h0, h1) in enumerate(AV_CHUNKS):
                            av = avp.tile([P, AV_MAXW], F32, tag="av")
                            for kt in range(KT):
                                nc.tensor.matmul(
                                    av[:, :h1 - h0],
                                    lhsT=ptb[:, kt, qt4 * P:(qt4 + 1) * P],
                                    rhs=v_sb[:, kt, h0:h1],
                                    start=(kt == 0),
                                    stop=(kt == KT - 1),
                                )
                            if ci == 0:
                                # l (row sums) is the last column (global idx D)
                                nc.vector.reciprocal(rl[:], av[:, D - h0:D - h0 + 1])
                            w_ = min(h1, D) - h0
                            nc.any.tensor_scalar_mul(
                                out=ob[:, h0:h0 + w_],
                                in0=av[:, :w_],
                                scalar1=rl[:],
                            )
                            if qb == QL // QB - 1 and qt4 == QB // P - 1:
                                # very last q-tile: stream the output per chunk
                                # so the final DMA isn't serialized behind all
                                # three normalizes (shaves the tail barrier)
                                nc.sync.dma_start(
                                    out=out[qrow:qrow + P, h0:h0 + w_],
                                    in_=ob[:, h0:h0 + w_],
                                )
                        if not (qb == QL // QB - 1 and qt4 == QB // P - 1):
                            nc.sync.dma_start(out=out[qrow:qrow + P, :], in_=ob[:])

    nc.finalize()
    return nc


def prepare_in_maps(q_embd, k_embd, v_embd, Wq, bq, Wk, bk, Wv, bv):
    bf16 = ml_dtypes.bfloat16
    f32 = np.float32

    def pack_x4(x):  # [B, L, D] f32 -> [B, P, QC, DT, XCH] bf16 (d = dt*P + p)
        xt = np.swapaxes(np.asarray(x, f32), 1, 2)  # [B, D, L]
        xt = xt.reshape(B, DT, P, QC, XCH).transpose(0, 2, 3, 1, 4)
        return np.ascontiguousarray(xt).astype(bf16)

    def pack_w(w):  # [D_in, D_out] f32 -> [P, DT, D_out] bf16
        return np.ascontiguousarray(
            np.asarray(w, f32).reshape(DT, P, -1).transpose(1, 0, 2)
        ).astype(bf16)

    xq4 = pack_x4(q_embd)
    xv4 = pack_x4(v_embd)
    # xk packed whole: [B, P, DT, KL]
    xkt = np.swapaxes(np.asarray(k_embd, f32), 1, 2).reshape(B, DT, P, KL)
    xkp = np.ascontiguousarray(xkt.transpose(0, 2, 1, 3)).astype(bf16)

    Wq = np.asarray(Wq, f32)
    Wk = np.asarray(Wk, f32)
    # M = Wq^T Wk (d1 x d2); c = (xk (Wk^T bq)) / 32 per batch/key
    m_ = pack_w(Wq.T @ Wk)
    u = Wk.T @ np.asarray(bq, f32)
    c = (np.asarray(k_embd, f32) @ u) * np.float32(SCALE)  # [B, KL]
    cb = np.ascontiguousarray(np.transpose(c.reshape(B, KT, P), (0, 2, 1)))  # [B,P,KT]
    wvp = pack_w(np.asarray(Wv, f32).T)
    bv_ = np.ascontiguousarray(np.asarray(bv, f32))

    return [
        {
            "xq4": xq4[i], "xkp": xkp[i], "xv4": xv4[i],
            "m": m_, "wvp": wvp, "cb": cb[i], "bv": bv_,
        }
        for i in range(NCORES)
    ]


_NC_CACHE = None


def get_nc() -> bass.Bass:
    global _NC_CACHE
    if _NC_CACHE is None:
        _NC_CACHE = build_bass()
    return _NC_CACHE


def run_on_device(in_maps, trace=False, **kwargs):
    return run_bass_kernel_spmd(get_nc(), in_maps, list(range(NCORES)), trace=trace, **kwargs)


def kernel(q_embd, k_embd, v_embd, Wq, bq, Wk, bk, Wv, bv):
    in_maps = prepare_in_maps(q_embd, k_embd, v_embd, Wq, bq, Wk, bk, Wv, bv)
    res = run_on_device(in_maps)
    return np.stack([r["out"] for r in res.results], axis=0)


# revision 18
# speedup vs baseline: 1.0008x; 1.0008x over previous
"""Fused multi-head-size-1 attention kernel for Trainium2 (Bass/Tile).

Problem: out = softmax((x_q Wq^T + bq)(x_k Wk^T + bk)^T / sqrt(D)) (x_v Wv^T + bv)
Shapes: B=8, QL=KL=2048, D=1024, fp32 in/out.

Sharding: data-parallel over batch. Core i processes batch i end-to-end;
no collectives.

Algebraic restructure (v2): expand the score product
  S = (xq Wq^T + 1 bq^T)(Wk xk^T + bk 1^T)
    = xq (Wq^T Wk) xk^T  +  (xq Wq^T bk) 1^T  +  1 (bq^T Wk xk^T)  +  (bq.bk) 1 1^T
The 2nd and 4th terms are constant along the key axis, so they drop out of
the softmax. Host precomputes M = Wq^T Wk (one 1024^3 matmul shared by all
batches) and c = (xk (Wk^T bq)) / 32 (a per-key bias folded into the exp
activation's per-partition bias operand). This removes the Q and K
projections entirely: device work drops from 30.1 to 25.8 GFLOP/core and
2032 -> 1792 matmul instructions.

Layout (v2.1): all DRAM operands are pre-packed host-side so every SBUF
partition's data is one contiguous DRAM run (8-32KB DMA descriptors; the
HWDGE queues are descriptor-rate-bound, so 2KB descriptors move at only
~110GB/s/queue while 8KB+ descriptors approach wire speed). A short PE
warm-up on scratch data during the DMA preamble absorbs the tensor
engine's p-state ramp.

Per-core dataflow (everything resident in SBUF in bf16):
  phase 1: XM^T[d2,q] = sum_d1 M[d1,d2]^T xq^T[d1,q]  (256 mm)
           V[k',h]   = xv Wv^T + bv, ones col appended (256 mm)
           xk^T is DMA'd straight to SBUF (no compute).
  phase 2: per q-block (512): S^T[k',q] = sum_d xk-tile^T XM^T (128 mm),
           P^T = exp(S^T/32 + c) via ScalarE (bf16, bias=c per partition),
           O[q,h] (+l) = P^T.T V_aug (192 mm, PSUM fp32),
           O = O * (1/l), DMA out. bv flows through the normalization
           (sum_k p_k/l = 1) so no epilogue bias is needed.
"""

import numpy as np
import ml_dtypes

import concourse.bass as bass
import concourse.mybir as mybir
from concourse.bacc import Bacc
from concourse.tile import TileContext
from concourse.bass_utils import run_bass_kernel_spmd

B, QL, KL, D = 8, 2048, 2048, 1024
P = 128
NCORES = 8
DT = D // P          # 8 tiles along d
KT = KL // P         # 16 tiles along k'
XCH = 512            # x streaming chunk along s
QC = QL // XCH       # 4 chunks
QB = 512             # q block for the attention stage
F32 = mybir.dt.float32
BF16 = mybir.dt.bfloat16
SCALE = 1.0 / 32.0   # 1/sqrt(D)

# AV free-dim chunking over V's 1025 columns (1024 h + ones column for l).
# The l-carrying chunk goes first so the reciprocal overlaps the other
# chunks' matmuls.
AV_CHUNKS = [(684, 1025), (0, 342), (342, 684)]
AV_MAXW = 342


def build_bass() -> bass.Bass:
    # Bacc (not bare Bass): its finalize() runs the pass pipeline that splits
    # multi-semaphore waits into event semaphores (TRN2 allows 1 wait/inst).
    nc = Bacc()

    # All operands pre-packed [partition, ...contiguous per partition].
    xq4 = nc.declare_dram_parameter("xq4", [P, QC, DT, XCH], BF16, isOutput=False)
    xkp = nc.declare_dram_parameter("xkp", [P, DT, KL], BF16, isOutput=False)
    xv4 = nc.declare_dram_parameter("xv4", [P, QC, DT, XCH], BF16, isOutput=False)
    m = nc.declare_dram_parameter("m", [P, DT, D], BF16, isOutput=False)
    wvp = nc.declare_dram_parameter("wvp", [P, DT, D], BF16, isOutput=False)
    cb = nc.declare_dram_parameter("cb", [P, KT], F32, isOutput=False)
    bv = nc.declare_dram_parameter("bv", [D], F32, isOutput=False)
    out = nc.declare_dram_parameter("out", [QL, D], F32, isOutput=True)

    with TileContext(nc) as tc:
        with (
            tc.tile_pool(name="persist", bufs=1) as persist,
            tc.tile_pool(name="consts", bufs=1) as consts,
        ):
            xk_sb = persist.tile([P, DT, KL], BF16, tag="xk")     # xk^T[d%128, dt, k]
            xmt_sb = persist.tile([P, DT, QL], BF16, tag="xmt")   # XM^T[d2%128, d2t, q]
            v_sb = persist.tile([P, KT, D + 1], BF16, tag="v")    # V[k'%128, kt, h|1]

            cb_sb = consts.tile([P, KT], F32, tag="cb")
            bv_sb = consts.tile([P, D], F32, tag="bv")
            warm = consts.tile([P, P], BF16, tag="warm")

            # ---------------- phase 1: XM + V projection ----------------
            with (
                tc.tile_pool(name="wpool", bufs=1) as wpool,
                tc.tile_pool(name="xpool", bufs=6) as xpool,
                tc.tile_pool(name="projp", bufs=8, space="PSUM") as projp,
            ):
                # PE warm-up on scratch zeros while the first DMAs land: the
                # tensor engine ramps 0.65 -> 1.2 -> 2.4 GHz over ~3us of
                # continuous work, so burn the ramp before real matmuls.
                nc.vector.memset(warm[:], 0.0)
                wps = projp.tile([P, XCH], F32, tag="proj")
                for _ in range(135):
                    nc.tensor.matmul(wps[:, 0:P], lhsT=warm[:], rhs=warm[:],
                                     start=True, stop=True)

                # Critical-path DMAs. Queue roles (measured rates differ a
                # lot, and descriptor size matters: <=4KB runs at half rate):
                # M split 5/8 on the fast gpsimd queue and 3/8 on scalar,
                # each as ONE big-descriptor DMA; xq0 in halves on SP's
                # queue. The slow scalar queue otherwise only carries data
                # needed late (Wv, cb). xpool bufs=6 so no x-chunk DMA
                # trigger ever blocks its engine waiting for a buffer.
                # M quartered across the two hardware-DGE queues (SP + ACT,
                # which spin up ~4us sooner than gpsimd's software DGE),
                # interleaved so each quarter lands before the d1t-outer
                # sweep consumes it.
                m_sb = wpool.tile([P, DT, D], BF16, tag="m")
                nc.sync.dma_start(out=m_sb[:, 0:2, :], in_=m[:, 0:2, :])
                nc.scalar.dma_start(out=m_sb[:, 2:4, :], in_=m[:, 2:4, :])
                nc.sync.dma_start(out=m_sb[:, 4:6, :], in_=m[:, 4:6, :])
                nc.scalar.dma_start(out=m_sb[:, 6:, :], in_=m[:, 6:, :])

                # XM^T: out[d2-tile, q-chunk] = sum_d1 M[d1,d2-tile]^T @ xq^T[d1,q-chunk]
                # d1t-outer with 8 concurrent PSUM groups: the PE consumes M
                # and xq slice-by-slice as they stream in. xq1 rides the same
                # queue as M's first half so a slow queue shifts both
                # together instead of opening a pipeline bubble.
                xcs = []
                for qc in range(QC):
                    xc = xpool.tile([P, DT, XCH], BF16, tag="x", name=f"xc{qc}")
                    xcs.append(xc)
                for dd in range(0, DT, 4):  # xq0 halves on gpsimd's SWDGE
                    nc.gpsimd.dma_start(out=xcs[0][:, dd:dd + 4, :],
                                        in_=xq4[:, 0, dd:dd + 4, :])
                nc.sync.dma_start(out=xcs[1][:], in_=xq4[:, 1, :, :])
                nc.sync.dma_start(out=xcs[2][:], in_=xq4[:, 2, :, :])
                nc.sync.dma_start(out=xcs[3][:], in_=xq4[:, 3, :, :])
                for qc in range(QC):
                    xc = xcs[qc]
                    pss = [projp.tile([P, XCH], F32, tag="proj", name=f"ps{qc}_{i}")
                           for i in range(DT)]
                    for d1t in range(DT):
                        for d2t in range(DT):
                            nc.tensor.matmul(
                                pss[d2t][:],
                                lhsT=m_sb[:, d1t, d2t * P:(d2t + 1) * P],
                                rhs=xc[:, d1t, :],
                                start=(d1t == 0),
                                stop=(d1t == DT - 1),
                            )
                    for d2t in range(DT):
                        nc.any.tensor_copy(
                            xmt_sb[:, d2t, qc * XCH:(qc + 1) * XCH], pss[d2t][:]
                        )

                # background DMAs for later stages, in need-order
                wv_sb = wpool.tile([P, DT, D], BF16, tag="wv")
                nc.scalar.dma_start(out=wv_sb[:], in_=wvp[:])
                nc.gpsimd.dma_start(out=xk_sb[:], in_=xkp[:])
                nc.scalar.dma_start(out=cb_sb[:], in_=cb[:])
                # broadcast bv across all partitions (stride-0 partition AP -> SWDGE)
                bv_bcast = bass.AP(tensor=bv[:].tensor, offset=0, ap=[[0, P], [1, D]])
                nc.gpsimd.dma_start(out=bv_sb[:], in_=bv_bcast)

                # V: out[s-tile, h-chunk] = sum_dt xv^T[d,s-tile]^T @ Wv^T[d,h-chunk]
                # + bv (broadcast over rows), fused into the PSUM->SBUF move.
                for cc in range(QC):
                    xc = xpool.tile([P, DT, XCH], BF16, tag="x")
                    nc.gpsimd.dma_start(out=xc[:], in_=xv4[:, cc, :, :])
                    for st4 in range(XCH // P):
                        st = cc * (XCH // P) + st4
                        for hc in range(D // 512):
                            ps = projp.tile([P, 512], F32, tag="proj")
                            for dt in range(DT):
                                nc.tensor.matmul(
                                    ps[:],
                                    lhsT=xc[:, dt, st4 * P:(st4 + 1) * P],
                                    rhs=wv_sb[:, dt, hc * 512:(hc + 1) * 512],
                                    start=(dt == 0),
                                    stop=(dt == DT - 1),
                                )
                            nc.any.tensor_add(
                                out=v_sb[:, st, hc * 512:(hc + 1) * 512],
                                in0=ps[:],
                                in1=bv_sb[:, hc * 512:(hc + 1) * 512],
                            )
                nc.vector.memset(v_sb[:, :, D], 1.0)  # ones column -> row sums

            # ---------------- phase 2: attention ----------------
            with (
                tc.tile_pool(name="ptpool", bufs=2) as ptpool,
                tc.tile_pool(name="opool", bufs=3) as opool,
                tc.tile_pool(name="small", bufs=4) as small,
                tc.tile_pool(name="scorep", bufs=3, space="PSUM") as scorep,
                tc.tile_pool(name="avp", bufs=4, space="PSUM") as avp,
            ):
                for qb in range(QL // QB):
                    q0 = qb * QB
                    ptb = ptpool.tile([P, KT, QB], BF16, tag="pt")
                    # scores S^T[k', q] = sum_dt xk-tile^T @ XM^T; exp folds
                    # in the per-key bias c (constant-per-query terms of the
                    # full score expansion drop out of softmax).
                    for kt in range(KT):
                        sp = scorep.tile([P, QB], F32, tag="score")
                        for dt in range(DT):
                            nc.tensor.matmul(
                                sp[:],
                                lhsT=xk_sb[:, dt, kt * P:(kt + 1) * P],
                                rhs=xmt_sb[:, dt, q0:q0 + QB],
                                start=(dt == 0),
                                stop=(dt == DT - 1),
                            )
                        nc.scalar.activation(
                            out=ptb[:, kt, :],
                            in_=sp[:],
                            func=mybir.ActivationFunctionType.Exp,
                            scale=SCALE,
                            bias=cb_sb[:, kt:kt + 1],
                        )
                    # AV + row sums + normalize, one q-tile (128 rows) at a time.
                    for qt4 in range(QB // P):
                        qrow = q0 + qt4 * P
                        rl = small.tile([P, 1], F32, tag="rl")
                        ob = opool.tile([P, D], F32, tag="o")
                        for ci, (h0, h1) in enumerate(AV_CHUNKS):
                            av = avp.tile([P, AV_MAXW], F32, tag="av")
                            for kt in range(KT):
                                nc.tensor.matmul(
                                    av[:, :h1 - h0],
                                    lhsT=ptb[:, kt, qt4 * P:(qt4 + 1) * P],
                                    rhs=v_sb[:, kt, h0:h1],
                                    start=(kt == 0),
                                    stop=(kt == KT - 1),
                                )
                            if ci == 0:
                                # l (row sums) is the last column (global idx D)
                                nc.vector.reciprocal(rl[:], av[:, D - h0:D - h0 + 1])
                            w_ = min(h1, D) - h0
                            nc.any.tensor_scalar_mul(
                                out=ob[:, h0:h0 + w_],
                                in0=av[:, :w_],
                                scalar1=rl[:],
                            )
                            if qb == QL // QB - 1 and qt4 == QB // P - 1:
                                # very last q-tile: stream the output per chunk
                                # so the final DMA isn't serialized behind all
                                # three normalizes (shaves the tail barrier)
                                nc.sync.dma_start(
                                    out=out[qrow:qrow + P, h0:h0 + w_],
                                    in_=ob[:, h0:h0 + w_],
                                )
                        if not (qb == QL // QB - 1 and qt4 == QB // P - 1):
                            nc.sync.dma_start(out=out[qrow:qrow + P, :], in_=ob[:])

    nc.finalize()
    return nc


def prepare_in_maps(q_embd, k_embd, v_embd, Wq, bq, Wk, bk, Wv, bv):
    bf16 = ml_dtypes.bfloat16
    f32 = np.float32

    def pack_x4(x):  # [B, L, D] f32 -> [B, P, QC, DT, XCH] bf16 (d = dt*P + p)
        xt = np.swapaxes(np.asarray(x, f32), 1, 2)  # [B, D, L]
        xt = xt.reshape(B, DT, P, QC, XCH).transpose(0, 2, 3, 1, 4)
        return np.ascontiguousarray(xt).astype(bf16)

    def pack_w(w):  # [D_in, D_out] f32 -> [P, DT, D_out] bf16
        return np.ascontiguousarray(
            np.asarray(w, f32).reshape(DT, P, -1).transpose(1, 0, 2)
        ).astype(bf16)

    xq4 = pack_x4(q_embd)
    xv4 = pack_x4(v_embd)
    # xk packed whole: [B, P, DT, KL]
    xkt = np.swapaxes(np.asarray(k_embd, f32), 1, 2).reshape(B, DT, P, KL)
    xkp = np.ascontiguousarray(xkt.transpose(0, 2, 1, 3)).astype(bf16)

    Wq = np.asarray(Wq, f32)
    Wk = np.asarray(Wk, f32)
    # M = Wq^T Wk (d1 x d2); c = (xk (Wk^T bq)) / 32 per batch/key
    m_ = pack_w(Wq.T @ Wk)
    u = Wk.T @ np.asarray(bq, f32)
    c = (np.asarray(k_embd, f32) @ u) * np.float32(SCALE)  # [B, KL]
    cb = np.ascontiguousarray(np.transpose(c.reshape(B, KT, P), (0, 2, 1)))  # [B,P,KT]
    wvp = pack_w(np.asarray(Wv, f32).T)
    bv_ = np.ascontiguousarray(np.asarray(bv, f32))

    return [
        {
            "xq4": xq4[i], "xkp": xkp[i], "xv4": xv4[i],
            "m": m_, "wvp": wvp, "cb": cb[i], "bv": bv_,
        }
        for i in range(NCORES)
    ]


_NC_CACHE = None


def get_nc() -> bass.Bass:
    global _NC_CACHE
    if _NC_CACHE is None:
        _NC_CACHE = build_bass()
    return _NC_CACHE


def run_on_device(in_maps, trace=False, **kwargs):
    return run_bass_kernel_spmd(get_nc(), in_maps, list(range(NCORES)), trace=trace, **kwargs)


def kernel(q_embd, k_embd, v_embd, Wq, bq, Wk, bk, Wv, bv):
    in_maps = prepare_in_maps(q_embd, k_embd, v_embd, Wq, bq, Wk, bk, Wv, bv)
    res = run_on_device(in_maps)
    return np.stack([r["out"] for r in res.results], axis=0)


# revision 19
# speedup vs baseline: 1.0011x; 1.0003x over previous
"""Fused multi-head-size-1 attention kernel for Trainium2 (Bass/Tile).

Problem: out = softmax((x_q Wq^T + bq)(x_k Wk^T + bk)^T / sqrt(D)) (x_v Wv^T + bv)
Shapes: B=8, QL=KL=2048, D=1024, fp32 in/out.

Sharding: data-parallel over batch. Core i processes batch i end-to-end;
no collectives.

Algebraic restructure (v2): expand the score product
  S = (xq Wq^T + 1 bq^T)(Wk xk^T + bk 1^T)
    = xq (Wq^T Wk) xk^T  +  (xq Wq^T bk) 1^T  +  1 (bq^T Wk xk^T)  +  (bq.bk) 1 1^T
The 2nd and 4th terms are constant along the key axis, so they drop out of
the softmax. Host precomputes M = Wq^T Wk (one 1024^3 matmul shared by all
batches) and c = (xk (Wk^T bq)) / 32 (a per-key bias folded into the exp
activation's per-partition bias operand). This removes the Q and K
projections entirely: device work drops from 30.1 to 25.8 GFLOP/core and
2032 -> 1792 matmul instructions.

Layout (v2.1): all DRAM operands are pre-packed host-side so every SBUF
partition's data is one contiguous DRAM run (8-32KB DMA descriptors; the
HWDGE queues are descriptor-rate-bound, so 2KB descriptors move at only
~110GB/s/queue while 8KB+ descriptors approach wire speed). A short PE
warm-up on scratch data during the DMA preamble absorbs the tensor
engine's p-state ramp.

Per-core dataflow (everything resident in SBUF in bf16):
  phase 1: XM^T[d2,q] = sum_d1 M[d1,d2]^T xq^T[d1,q]  (256 mm)
           V[k',h]   = xv Wv^T + bv, ones col appended (256 mm)
           xk^T is DMA'd straight to SBUF (no compute).
  phase 2: per q-block (512): S^T[k',q] = sum_d xk-tile^T XM^T (128 mm),
           P^T = exp(S^T/32 + c) via ScalarE (bf16, bias=c per partition),
           O[q,h] (+l) = P^T.T V_aug (192 mm, PSUM fp32),
           O = O * (1/l), DMA out. bv flows through the normalization
           (sum_k p_k/l = 1) so no epilogue bias is needed.
"""

import numpy as np
import ml_dtypes

import concourse.bass as bass
import concourse.mybir as mybir
from concourse.bacc import Bacc
from concourse.tile import TileContext
from concourse.bass_utils import run_bass_kernel_spmd

B, QL, KL, D = 8, 2048, 2048, 1024
P = 128
NCORES = 8
DT = D // P          # 8 tiles along d
KT = KL // P         # 16 tiles along k'
XCH = 512            # x streaming chunk along s
QC = QL // XCH       # 4 chunks
QB = 512             # q block for the attention stage
F32 = mybir.dt.float32
BF16 = mybir.dt.bfloat16
SCALE = 1.0 / 32.0   # 1/sqrt(D)

# AV free-dim chunking over V's 1025 columns (1024 h + ones column for l).
# The l-carrying chunk goes first so the reciprocal overlaps the other
# chunks' matmuls.
AV_CHUNKS = [(684, 1025), (0, 342), (342, 684)]
AV_MAXW = 342


def build_bass() -> bass.Bass:
    # Bacc (not bare Bass): its finalize() runs the pass pipeline that splits
    # multi-semaphore waits into event semaphores (TRN2 allows 1 wait/inst).
    nc = Bacc()

    # All operands pre-packed [partition, ...contiguous per partition].
    xq4 = nc.declare_dram_parameter("xq4", [P, QC, DT, XCH], BF16, isOutput=False)
    xkp = nc.declare_dram_parameter("xkp", [P, DT, KL], BF16, isOutput=False)
    xv4 = nc.declare_dram_parameter("xv4", [P, QC, DT, XCH], BF16, isOutput=False)
    m = nc.declare_dram_parameter("m", [P, DT, D], BF16, isOutput=False)
    wvp = nc.declare_dram_parameter("wvp", [P, DT, D], BF16, isOutput=False)
    cb = nc.declare_dram_parameter("cb", [P, KT], F32, isOutput=False)
    bv = nc.declare_dram_parameter("bv", [D], F32, isOutput=False)
    out = nc.declare_dram_parameter("out", [QL, D], F32, isOutput=True)

    with TileContext(nc) as tc:
        with (
            tc.tile_pool(name="persist", bufs=1) as persist,
            tc.tile_pool(name="consts", bufs=1) as consts,
        ):
            xk_sb = persist.tile([P, DT, KL], BF16, tag="xk")     # xk^T[d%128, dt, k]
            xmt_sb = persist.tile([P, DT, QL], BF16, tag="xmt")   # XM^T[d2%128, d2t, q]
            v_sb = persist.tile([P, KT, D + 1], BF16, tag="v")    # V[k'%128, kt, h|1]

            cb_sb = consts.tile([P, KT], F32, tag="cb")
            bv_sb = consts.tile([P, D], F32, tag="bv")
            warm = consts.tile([P, P], BF16, tag="warm")

            # ---------------- phase 1: XM + V projection ----------------
            with (
                tc.tile_pool(name="wpool", bufs=1) as wpool,
                tc.tile_pool(name="xpool", bufs=6) as xpool,
                tc.tile_pool(name="projp", bufs=8, space="PSUM") as projp,
            ):
                # PE warm-up on scratch zeros while the first DMAs land: the
                # tensor engine ramps 0.65 -> 1.2 -> 2.4 GHz over ~3us of
                # continuous work, so burn the ramp before real matmuls.
                nc.vector.memset(warm[:], 0.0)
                wps = projp.tile([P, XCH], F32, tag="proj")
                for _ in range(135):
                    nc.tensor.matmul(wps[:, 0:P], lhsT=warm[:], rhs=warm[:],
                                     start=True, stop=True)

                # Critical-path DMAs. Queue roles (measured rates differ a
                # lot, and descriptor size matters: <=4KB runs at half rate):
                # M split 5/8 on the fast gpsimd queue and 3/8 on scalar,
                # each as ONE big-descriptor DMA; xq0 in halves on SP's
                # queue. The slow scalar queue otherwise only carries data
                # needed late (Wv, cb). xpool bufs=6 so no x-chunk DMA
                # trigger ever blocks its engine waiting for a buffer.
                # M quartered across the two hardware-DGE queues (SP + ACT,
                # which spin up ~4us sooner than gpsimd's software DGE),
                # interleaved so each quarter lands before the d1t-outer
                # sweep consumes it.
                m_sb = wpool.tile([P, DT, D], BF16, tag="m")
                nc.sync.dma_start(out=m_sb[:, 0:2, :], in_=m[:, 0:2, :])
                nc.scalar.dma_start(out=m_sb[:, 2:4, :], in_=m[:, 2:4, :])
                nc.sync.dma_start(out=m_sb[:, 4:6, :], in_=m[:, 4:6, :])
                nc.scalar.dma_start(out=m_sb[:, 6:, :], in_=m[:, 6:, :])

                # XM^T: out[d2-tile, q-chunk] = sum_d1 M[d1,d2-tile]^T @ xq^T[d1,q-chunk]
                # d1t-outer with 8 concurrent PSUM groups: the PE consumes M
                # and xq slice-by-slice as they stream in. xq1 rides the same
                # queue as M's first half so a slow queue shifts both
                # together instead of opening a pipeline bubble.
                xcs = []
                for qc in range(QC):
                    xc = xpool.tile([P, DT, XCH], BF16, tag="x", name=f"xc{qc}")
                    xcs.append(xc)
                for dd in range(0, DT, 4):  # xq0 halves on gpsimd's SWDGE
                    nc.gpsimd.dma_start(out=xcs[0][:, dd:dd + 4, :],
                                        in_=xq4[:, 0, dd:dd + 4, :])
                nc.sync.dma_start(out=xcs[1][:], in_=xq4[:, 1, :, :])
                nc.sync.dma_start(out=xcs[2][:], in_=xq4[:, 2, :, :])
                nc.sync.dma_start(out=xcs[3][:], in_=xq4[:, 3, :, :])
                for qc in range(QC):
                    xc = xcs[qc]
                    pss = [projp.tile([P, XCH], F32, tag="proj", name=f"ps{qc}_{i}")
                           for i in range(DT)]
                    for d1t in range(DT):
                        for d2t in range(DT):
                            nc.tensor.matmul(
                                pss[d2t][:],
                                lhsT=m_sb[:, d1t, d2t * P:(d2t + 1) * P],
                                rhs=xc[:, d1t, :],
                                start=(d1t == 0),
                                stop=(d1t == DT - 1),
                            )
                    for d2t in range(DT):
                        nc.any.tensor_copy(
                            xmt_sb[:, d2t, qc * XCH:(qc + 1) * XCH], pss[d2t][:]
                        )

                # background DMAs for later stages, in need-order
                wv_sb = wpool.tile([P, DT, D], BF16, tag="wv")
                nc.scalar.dma_start(out=wv_sb[:], in_=wvp[:])
                nc.gpsimd.dma_start(out=xk_sb[:], in_=xkp[:])
                nc.scalar.dma_start(out=cb_sb[:], in_=cb[:])
                # broadcast bv across all partitions (stride-0 partition AP -> SWDGE)
                bv_bcast = bass.AP(tensor=bv[:].tensor, offset=0, ap=[[0, P], [1, D]])
                nc.gpsimd.dma_start(out=bv_sb[:], in_=bv_bcast)

                # V: out[s-tile, h-chunk] = sum_dt xv^T[d,s-tile]^T @ Wv^T[d,h-chunk]
                # + bv (broadcast over rows), fused into the PSUM->SBUF move.
                for cc in range(QC):
                    xc = xpool.tile([P, DT, XCH], BF16, tag="x")
                    nc.gpsimd.dma_start(out=xc[:], in_=xv4[:, cc, :, :])
                    for st4 in range(XCH // P):
                        st = cc * (XCH // P) + st4
                        for hc in range(D // 512):
                            ps = projp.tile([P, 512], F32, tag="proj")
                            for dt in range(DT):
                                nc.tensor.matmul(
                                    ps[:],
                                    lhsT=xc[:, dt, st4 * P:(st4 + 1) * P],
                                    rhs=wv_sb[:, dt, hc * 512:(hc + 1) * 512],
                                    start=(dt == 0),
                                    stop=(dt == DT - 1),
                                )
                            nc.any.tensor_add(
                                out=v_sb[:, st, hc * 512:(hc + 1) * 512],
                                in0=ps[:],
                                in1=bv_sb[:, hc * 512:(hc + 1) * 512],
                            )
                nc.vector.memset(v_sb[:, :, D], 1.0)  # ones column -> row sums

            # ---------------- phase 2: attention ----------------
            with (
                tc.tile_pool(name="ptpool", bufs=2) as ptpool,
                tc.tile_pool(name="opool", bufs=3) as opool,
                tc.tile_pool(name="small", bufs=4) as small,
                tc.tile_pool(name="scorep", bufs=4, space="PSUM") as scorep,
                tc.tile_pool(name="avp", bufs=4, space="PSUM") as avp,
            ):
                for qb in range(QL // QB):
                    q0 = qb * QB
                    ptb = ptpool.tile([P, KT, QB], BF16, tag="pt")
                    # scores S^T[k', q] = sum_dt xk-tile^T @ XM^T; exp folds
                    # in the per-key bias c (constant-per-query terms of the
                    # full score expansion drop out of softmax).
                    for kt in range(KT):
                        sp = scorep.tile([P, QB], F32, tag="score")
                        for dt in range(DT):
                            nc.tensor.matmul(
                                sp[:],
                                lhsT=xk_sb[:, dt, kt * P:(kt + 1) * P],
                                rhs=xmt_sb[:, dt, q0:q0 + QB],
                                start=(dt == 0),
                                stop=(dt == DT - 1),
                            )
                        nc.scalar.activation(
                            out=ptb[:, kt, :],
                            in_=sp[:],
                            func=mybir.ActivationFunctionType.Exp,
                            scale=SCALE,
                            bias=cb_sb[:, kt:kt + 1],
                        )
                    # AV + row sums + normalize, one q-tile (128 rows) at a time.
                    for qt4 in range(QB // P):
                        qrow = q0 + qt4 * P
                        rl = small.tile([P, 1], F32, tag="rl")
                        ob = opool.tile([P, D], F32, tag="o")
                        for ci, (h0, h1) in enumerate(AV_CHUNKS):
                            av = avp.tile([P, AV_MAXW], F32, tag="av")
                            for kt in range(KT):
                                nc.tensor.matmul(
                                    av[:, :h1 - h0],
                                    lhsT=ptb[:, kt, qt4 * P:(qt4 + 1) * P],
                                    rhs=v_sb[:, kt, h0:h1],
                                    start=(kt == 0),
                                    stop=(kt == KT - 1),
                                )
                            if ci == 0:
                                # l (row sums) is the last column (global idx D)
                                nc.vector.reciprocal(rl[:], av[:, D - h0:D - h0 + 1])
                            w_ = min(h1, D) - h0
                            nc.any.tensor_scalar_mul(
                                out=ob[:, h0:h0 + w_],
                                in0=av[:, :w_],
                                scalar1=rl[:],
                            )
                            if qb == QL // QB - 1 and qt4 == QB // P - 1:
                                # very last q-tile: stream the output per chunk
                                # so the final DMA isn't serialized behind all
                                # three normalizes (shaves the tail barrier)
                                nc.sync.dma_start(
                                    out=out[qrow:qrow + P, h0:h0 + w_],
                                    in_=ob[:, h0:h0 + w_],
                                )
                        if not (qb == QL // QB - 1 and qt4 == QB // P - 1):
                            nc.sync.dma_start(out=out[qrow:qrow + P, :], in_=ob[:])

    nc.finalize()
    return nc


def prepare_in_maps(q_embd, k_embd, v_embd, Wq, bq, Wk, bk, Wv, bv):
    bf16 = ml_dtypes.bfloat16
    f32 = np.float32

    def pack_x4(x):  # [B, L, D] f32 -> [B, P, QC, DT, XCH] bf16 (d = dt*P + p)
        xt = np.swapaxes(np.asarray(x, f32), 1, 2)  # [B, D, L]
        xt = xt.reshape(B, DT, P, QC, XCH).transpose(0, 2, 3, 1, 4)
        return np.ascontiguousarray(xt).astype(bf16)

    def pack_w(w):  # [D_in, D_out] f32 -> [P, DT, D_out] bf16
        return np.ascontiguousarray(
            np.asarray(w, f32).reshape(DT, P, -1).transpose(1, 0, 2)
        ).astype(bf16)

    xq4 = pack_x4(q_embd)
    xv4 = pack_x4(v_embd)
    # xk packed whole: [B, P, DT, KL]
    xkt = np.swapaxes(np.asarray(k_embd, f32), 1, 2).reshape(B, DT, P, KL)
    xkp = np.ascontiguousarray(xkt.transpose(0, 2, 1, 3)).astype(bf16)

    Wq = np.asarray(Wq, f32)
    Wk = np.asarray(Wk, f32)
    # M = Wq^T Wk (d1 x d2); c = (xk (Wk^T bq)) / 32 per batch/key
    m_ = pack_w(Wq.T @ Wk)
    u = Wk.T @ np.asarray(bq, f32)
    c = (np.asarray(k_embd, f32) @ u) * np.float32(SCALE)  # [B, KL]
    cb = np.ascontiguousarray(np.transpose(c.reshape(B, KT, P), (0, 2, 1)))  # [B,P,KT]
    wvp = pack_w(np.asarray(Wv, f32).T)
    bv_ = np.ascontiguousarray(np.asarray(bv, f32))

    return [
        {
            "xq4": xq4[i], "xkp": xkp[i], "xv4": xv4[i],
            "m": m_, "wvp": wvp, "cb": cb[i], "bv": bv_,
        }
        for i in range(NCORES)
    ]


_NC_CACHE = None


def get_nc() -> bass.Bass:
    global _NC_CACHE
    if _NC_CACHE is None:
        _NC_CACHE = build_bass()
    return _NC_CACHE


def run_on_device(in_maps, trace=False, **kwargs):
    return run_bass_kernel_spmd(get_nc(), in_maps, list(range(NCORES)), trace=trace, **kwargs)


def kernel(q_embd, k_embd, v_embd, Wq, bq, Wk, bk, Wv, bv):
    in_maps = prepare_in_maps(q_embd, k_embd, v_embd, Wq, bq, Wk, bk, Wv, bv)
    res = run_on_device(in_maps)
    return np.stack([r["out"] for r in res.results], axis=0)
